# revision 4
# baseline (speedup 1.0000x reference)
"""Row-normalize block-diagonal graph weights on 8 Trainium2 NeuronCores.

fp16 I/O (rel-err budget 2e-2 dwarfs fp16 rounding ~5e-4): host downcasts
edge_weight, device streams 8MB in + 8MB out per core, host upcasts.

Per-core pipeline over 10 chunks (2,2,4,4,4,4,4,4,2,2 graph-row columns):
  SP ring:  all loads (single HWDGE queue; splitting loads across queues
            measurably LOWERS union bandwidth on this part)
  DVE:      per-chunk f32 row sums (TENSOR_REDUCE) + corr add + 1/x
  ACT:      per-chunk normalize multiply, then pushes its own store on
            the second HWDGE queue
  Pool:     corr load only (block exits with no_gpsimd_drain)
Small first chunks shorten pipeline fill; small last chunks shorten the
exposed mul+store tail. The zero-degree clamp lives on the host (rows
with non-positive degree are routed through the exact fixup path).

Sharding: pure data parallel over K — each core owns 4 graphs
([4096, 1024] slab); no cross-core communication.
"""

import numpy as np

K = 32          # graphs in batch
N = 1024        # nodes per graph
NCORES = 8
KPC = K // NCORES          # graphs per core
ROWS = KPC * N             # 4096 source-node rows per core
NODES = K * N              # total segments
P = 128                    # SBUF partitions
Q = 4                      # consecutive rows per partition per slab
T = ROWS // (Q * P)        # 8 slabs per core

_CACHE = {}


def _build_bass():
    if "nc" in _CACHE:
        return _CACHE["nc"]

    import concourse.bass as bass
    from concourse import mybir

    f32 = mybir.dt.float32
    f16 = mybir.dt.float16
    nc = bass.Bass("TRN2", target_bir_lowering=False, debug=False,
                   num_devices=NCORES)
    x = nc.dram_tensor("x", [ROWS, N], f16, kind="ExternalInput").ap()
    corr = nc.dram_tensor("corr", [P, T * Q], f32, kind="ExternalInput").ap()
    y = nc.dram_tensor("y", [ROWS, N], f16, kind="ExternalOutput").ap()
    # slab t covers rows [t*P*Q, (t+1)*P*Q): partition p holds Q
    # consecutive DRAM rows -> one contiguous (Q*N*2)B run per partition
    xt = x.rearrange("(t p q) n -> t p (q n)", p=P, q=Q)
    yt = y.rearrange("(t p q) n -> t p (q n)", p=P, q=Q)

    from contextlib import ExitStack
    with (
        nc.sbuf_tensor([P, T * Q * N], f16) as wall,
        nc.sbuf_tensor([P, T * Q], f32) as call_,
        nc.sbuf_tensor([P, T * Q], f32) as degall,
        nc.sbuf_tensor([P, T * Q], f32) as invall,
        nc.semaphore("s_corr") as s_corr,
        nc.semaphore("s_dn") as s_dn,
        ExitStack() as _sems,
        nc.Block(no_gpsimd_drain=True) as block,
    ):
        M = Q * N
        wap, cap = wall.ap(), call_.ap()
        degap, invap = degall.ap(), invall.ap()

        # (slab, q0, qc): small chunks at both ends of the pipeline
        chunks = ([(0, 0, 2), (0, 2, 2)]
                  + [(t, 0, 4) for t in range(1, 7)]
                  + [(7, 0, 2), (7, 2, 2)])
        NDVE = 4                   # chunks whose sums stay on the DVE
        s_in = [_sems.enter_context(nc.semaphore(f"s_ld{i}"))
                for i in range(len(chunks))]
        s_sum = [_sems.enter_context(nc.semaphore(f"s_sm{i}"))
                 for i in range(len(chunks))]
        s_st = [_sems.enter_context(nc.semaphore(f"s_st{i}"))
                for i in range(len(chunks))]

        def wslice(t, q0, qc):
            base = t * M + q0 * N
            return wap[:, base:base + qc * N]

        def sslice(ap_, t, q0, qc):
            base = t * Q + q0
            return ap_[:, base:base + qc]

        @block.sync
        def _(sync):
            for i, (t, q0, qc) in enumerate(chunks):
                sync.dma_start(out=wslice(t, q0, qc),
                               in_=xt[t][:, q0 * N:(q0 + qc) * N]
                               ).then_inc(s_in[i], 16)

        @block.vector
        def _(vector):
            # DVE: sums for the first NDVE chunks, then the degree
            # chain + the normalize multiply for EVERY chunk (16-bit
            # tensor_scalar runs ~3x faster here than ACT's activation)
            vector.wait_ge(s_corr, 16)
            for i, (t, q0, qc) in enumerate(chunks):
                if i < NDVE:
                    vector.wait_ge(s_in[i], 16)
                    for q in range(q0, q0 + qc):
                        col = t * Q + q
                        vector.reduce_sum(out=degap[:, col:col + 1],
                                          in_=wap[:, col * N:(col + 1) * N],
                                          axis=mybir.AxisListType.X)
                    # DVE is a deep pipeline without interlocks: drain
                    # between same-engine RAW-dependent ops
                    vector.drain()
                else:
                    vector.wait_ge(s_sum[i], 1)
                d = sslice(degap, t, q0, qc)
                vector.tensor_add(d, d, sslice(cap, t, q0, qc))
                vector.drain()
                vector.reciprocal(out=sslice(invap, t, q0, qc), in_=d)
                vector.drain()
                for q in range(q0, q0 + qc):
                    col = t * Q + q
                    vector.tensor_scalar_mul(
                        wap[:, col * N:(col + 1) * N],
                        wap[:, col * N:(col + 1) * N],
                        invap[:, col:col + 1])
                # drain before signalling: the muls' SBUF writes must
                # be visible to the SDMA engines before the store
                vector.drain().then_inc(s_st[i], 1)

        @block.scalar
        def _(scalar):
            # ACT: accum-sums for the later chunks, and every store
            # dispatch on its HWDGE ring, interleaved so each store
            # goes out as soon as the DVE's muls for it are visible
            def disp(i):
                t, q0, qc = chunks[i]
                scalar.wait_ge(s_st[i], 1)
                scalar.dma_start(out=yt[t][:, q0 * N:(q0 + qc) * N],
                                 in_=wslice(t, q0, qc)).then_inc(s_dn, 16)

            disp(0)
            disp(1)
            for i in range(NDVE, len(chunks)):
                t, q0, qc = chunks[i]
                scalar.wait_ge(s_in[i], 16)
                for q in range(q0, q0 + qc):
                    col = t * Q + q
                    scalar.activation(
                        wap[:, col * N:(col + 1) * N],
                        wap[:, col * N:(col + 1) * N],
                        mybir.ActivationFunctionType.Copy,
                        accum_out=degap[:, col:col + 1])
                scalar.drain().then_inc(s_sum[i], 1)
                disp(i - 2)
            disp(len(chunks) - 2)
            disp(len(chunks) - 1)
            scalar.wait_ge(s_dn, 16 * len(chunks))

        @block.gpsimd
        def _(gpsimd):
            # tiny contiguous-2D corr load on the (idle-at-start) PL
            # queue so it cannot clog the SP ring ahead of the big loads
            gpsimd.dma_start(out=cap[:, :], in_=corr).then_inc(s_corr, 16)

    _CACHE["nc"] = nc
    return nc


def _expected_row_pattern():
    if "base" not in _CACHE:
        _CACHE["base"] = (np.arange(K * N * N, dtype=np.int64) // N)
    return _CACHE["base"]


def _install_ntff_hook():
    """Recreate the NTFF profile hook the boot shim couldn't install
    (this image's antenv lacks axon_hooks). Safe no-op on failure."""
    import sys, types
    if "antenv.axon_hooks" in sys.modules:
        return
    try:
        from trn_agent_boot.trn_boot import _ntff_profile_via_ctypes
        hook = _ntff_profile_via_ctypes("/opt/axon/libaxon_pjrt.so")
        mod = types.ModuleType("antenv.axon_hooks")
        mod.get_axon_ntff_profile_hook = lambda: hook
        mod.set_axon_ntff_profile_hook = lambda h: None
        sys.modules["antenv.axon_hooks"] = mod
    except Exception:
        pass


def _run_spmd(edge_weight, corr, trace=False):
    from concourse.bass_utils import run_bass_kernel_spmd

    if trace:
        _install_ntff_hook()
    nc = _build_bass()
    ew = np.asarray(edge_weight)
    ew16 = np.ascontiguousarray(ew.astype(np.float16))
    corr = np.ascontiguousarray(np.asarray(corr, dtype=np.float32))
    cperm = corr.reshape(NCORES, T, P, Q).transpose(0, 2, 1, 3) \
               .reshape(NCORES, P, T * Q)
    in_maps = [{"x": ew16[c * KPC:(c + 1) * KPC].reshape(ROWS, N),
                "corr": np.ascontiguousarray(cperm[c])}
               for c in range(NCORES)]
    res = run_bass_kernel_spmd(nc, in_maps, list(range(NCORES)), trace=trace)
    out = np.empty((K, N * N), dtype=np.float32)
    for c in range(NCORES):
        out[c * KPC:(c + 1) * KPC] = \
            res.results[c]["y"].astype(np.float32).reshape(KPC, N * N)
    return out, res


def _prepare(edge_weight, row):
    """Host-side exact handling of E = {e : row[e] != e//N} plus the
    zero/negative-degree guard (the device applies no clamp).

    Returns (corr[NODES] f32 to add to the device row-sums,
             fixup_idx int64, fixup_val f32) so that
    rowsum+corr == segment_sum(w, row) and out[fixup_idx] = fixup_val
    reproduces deg_inv[clamped row] * w for every element whose exact
    value the device cannot produce.
    """
    w = edge_weight.reshape(-1)
    base = _expected_row_pattern()
    row = row.astype(np.int64, copy=False)
    E = np.flatnonzero(row != base)
    corr = np.zeros(NODES, dtype=np.float64)
    if E.size:
        wE = w[E].astype(np.float64)
        np.subtract.at(corr, base[E], wE)
        rE = row[E]
        valid = (rE >= 0) & (rE < NODES)
        np.add.at(corr, rE[valid], wE[valid])
    # accurate degrees for the fixup values
    deg = edge_weight.reshape(NODES, N).sum(axis=1, dtype=np.float64) + corr
    deg = deg.astype(np.float32)
    inv = np.where(deg > 0, np.float32(1.0) / deg, np.float32(0.0))
    # rows whose true degree is ~0 would hit the unclamped device 1/x:
    # route every element of such rows through the exact fixup instead
    bad = np.flatnonzero(deg < np.float32(1e-3))
    if bad.size:
        elems = (bad[:, None] * N + np.arange(N)[None, :]).reshape(-1)
        E = np.unique(np.concatenate([E, elems]))
    if E.size:
        gather = np.clip(row[E], 0, NODES - 1)   # jnp OOB gather clamps
        fixup_val = (w[E] * inv[gather]).astype(np.float32)
    else:
        fixup_val = np.zeros(0, dtype=np.float32)
    return corr.astype(np.float32), E, fixup_val


def kernel(edge_weight, row, num_atom):
    edge_weight = np.asarray(edge_weight)
    row = np.asarray(row)
    if (edge_weight.shape != (K, N * N)
            or int(num_atom) != N
            or row.shape != (K * N * N,)):
        return _numpy_reference(edge_weight, row, int(num_atom))
    corr, E, fixup_val = _prepare(edge_weight, row)
    out, _ = _run_spmd(edge_weight, corr)
    if E.size:
        out.reshape(-1)[E] = fixup_val
    return out


def _numpy_reference(edge_weight, row, num_atom):
    """jnp-semantics fallback for unexpected shapes: scatter drops OOB,
    gather clamps."""
    Kb = edge_weight.shape[0]
    num_nodes = Kb * num_atom
    w = edge_weight.reshape(-1).astype(np.float32)
    row = row.astype(np.int64, copy=False)
    valid = (row >= 0) & (row < num_nodes)
    deg = np.zeros(num_nodes, dtype=np.float64)
    np.add.at(deg, row[valid], w[valid].astype(np.float64))
    deg = deg.astype(np.float32)
    deg_inv = np.where(deg > 0, np.float32(1.0) / deg, np.float32(0.0))
    out = deg_inv[np.clip(row, 0, num_nodes - 1)] * w
    return out.reshape(Kb, -1).astype(np.float32)


def bench(edge_weight, row, num_atom, trace=True):
    """Like kernel() but returns (output, BassKernelResults) with profiling."""
    edge_weight = np.asarray(edge_weight)
    row = np.asarray(row)
    corr, E, fixup_val = _prepare(edge_weight, row)
    out, res = _run_spmd(edge_weight, corr, trace=trace)
    if E.size:
        out.reshape(-1)[E] = fixup_val
    return out, res
